# revision 58
# baseline (speedup 1.0000x reference)
"""FFM layer kernel for Trainium2, data-parallel over batch on 8 NeuronCores.

The reference computes, for each sample b:
    x = [dense(13) | onehot(26 fields x 1000)]            # [B, 26013]
    linear = w0 + x @ w                                   # [B, 1]
    field_f = einsum('bf,fik->bik', x, v)                 # [B, 39, 8]
    inter = 0.5*((sum_i field_f)^2.sum(k) - (field_f^2).sum(i,k))
    out = linear + inter

Because x is one-hot in the sparse block, x @ [v|w] is a 26-row gather from
an int8 [26013, 512] table (cols 0..311 = k-major flattened v row / step,
col 312 = w / step, rest pad to a 256 B multiple) plus a tiny fp32 dense
[14]x[14,314] matmul in 1/step units (vdx pre-divided by step on the host;
row 13 = ones row carrying w0/step into col 312).  Each core handles 512
samples as 4 tiles of 128.

Measured HW behavior that shaped the design: SWDGE desc-gen costs ~8.5 ns
per descriptor (+~1 us fixed per call) with ~4 concurrent per-queue lanes,
and a call's packets only flow once ITS generation completes - so the
gather is desc-GEN limited, not drain limited, and the call schedule
(CALL_SCHED) hand-balances fields per queue with tile 0 on all 4 lanes in
round 1.  Startup is dominated by the gpsimd mlp-library reload (~10.6 us,
fixed) which gates the first gather; the mega input DMA and dense matmuls
hide under it.  The int8 table (4-sigma clipped step; rel_fro ~1.1e-2 vs
the 2e-2 gate) keeps wire bytes at 512 B/row vs fp16's 768 B.

The 26 gathered rows are summed with a pairwise tree on DVE.  Level-1 of
half 0 is pre-converted int8->fp16 by the otherwise-idle Act engine (TRN2
DVE reads int8 at ~0.9 elem/ns vs fp16 2x ~1.65), half 1 runs int8 directly
(first, so the convert hides behind it); upper levels run fp16 2x.
Everything stays in integer "1/step" units until the very end: tot' = u +
dnt', h1' = act-square-accum(tot'), r = (h2'-h1') * 0.5*step^2, out =
tot'[312]*step + r.  The [128, 4] per-tile outputs are stored directly and
un-permuted on the host (no PE transpose pass).

Measured: 67.7-68.5 us warm (from an 86.1 us fp16 baseline; cold first
exec after a fresh compile reads several us higher). Structure: reload
~17.6 + gather-gen-limited window + gapless DVE ~32.5 us from ~29.5 +
store ~3.7.
"""

import os

import numpy as np

N_DENSE = 13
N_SPARSE = 26
ONEHOT = 1000
FIELD = 39
K = 8
FEAT = N_DENSE + N_SPARSE * ONEHOT  # 26013
B = 4096
NCORES = 8
BC = B // NCORES  # 512 samples per core
P = 128
NT = BC // P  # 4 tiles per core
D = FIELD * K  # 312
DW = D + 1  # 313 (col 312 carries the linear weight)
DP = D + 2  # 314 (even payload width; col 313 zero pad)
NI = N_SPARSE * P  # 3328 gathered rows per tile
HC = (NI // 2) // 16  # idx columns per 13-field half (104)

TABLE_F16 = os.environ.get("K_F16", "0") == "1"
CLIP_SIGMA = float(os.environ.get("K_CLIP", "4.0"))
K_SCR = int(os.environ.get("K_SCR", "65536"))  # SWDGE desc ring carveout
E = 384 if TABLE_F16 else 512  # gathered row width in elements (=bytes for i8)
# SWDGE desc-gen runs at ~8.5 ns/descriptor + ~1 us fixed per call, with 4
# concurrent per-queue lanes; a call's packets only flow once ITS generation
# finishes. (7,6) per half for tiles 0-2 puts tile 0 on all 4 lanes at once
# (early DVE start) and minimizes per-call fixed cost; the LAST tile uses
# (6,6,1) so its L1 pair-adds only need the two 6-field calls and the lone
# 13th field feeds a late op in the tree - the post-gather tail chain
# shrinks from ~7 us to ~3.5 us.
SPLITS_T = (
    [((0, 5), (5, 4), (9, 4))]
    + [((0, 7), (7, 6))] * 2
    + [((0, 6), (6, 6), (12, 1))]
)
# calls whose rows feed the L1 pair adds (rows 0..11)
L1SET_T = [(0, 1, 2)] + [(0, 1)] * 3
ROW12CALL_T = [2, 1, 1, 2]  # call index holding field 12, per tile
# explicit (tile, half, call) issue order with hand-balanced queues
# (26 fields per queue, dispatch order matches lane-free order). Tile 0 is
# fine-split (5,4,4) so its six calls pipeline through the lanes and the
# DVE can start ~1.4 us earlier; the schedule tail is fine-grained -
# ending with a full coarse round measured +3.8 us of window.
CALL_SCHED = [
    (0, 0, 0, 0), (0, 1, 0, 1), (0, 0, 1, 2), (0, 1, 1, 3),
    (0, 0, 2, 2), (0, 1, 2, 3),
    (1, 0, 0, 0), (1, 1, 0, 1), (1, 0, 1, 2), (1, 1, 1, 3),
    (2, 0, 0, 0), (2, 1, 0, 1), (2, 0, 1, 2), (2, 1, 1, 3),
    (3, 0, 0, 0), (3, 1, 0, 1), (3, 0, 1, 2), (3, 1, 1, 3),
    (3, 0, 2, 0), (3, 1, 2, 1),
]
K_CONV = os.environ.get("K_CONV", "1") == "1"  # Act converts h0 rows to fp16
# h1 runs on DVE for the first DVE_H1 tiles, on Act for the rest.
# DVE_H1=2 should kill the early-tile r-op stalls but measured net-slower
# (71.7us vs 69.3us); 0 is the measured best.
DVE_H1 = int(os.environ.get("K_DVEH1", "0"))
TILE_VOPS = [14 if t < DVE_H1 else 12 for t in range(NT)]
TBASE = [sum(TILE_VOPS[:t]) for t in range(NT)]

# mega input layout (bytes per partition)
MB_IDX = 0                    # [128, 832] int16 = 1664 B
MB_DNT = 1664                 # [14, 512] fp32 = 2048 B
MB_VDX = MB_DNT + 2048        # [14, 314] fp32 = 1256 B
MB = MB_VDX + 1256            # 4968

_cached = {}


def _build_program(step: float):
    key = step
    if key in _cached:
        return _cached[key]

    import concourse.bacc as bacc
    import concourse.mybir as mybir

    nc = bacc.Bacc(
        "TRN2",
        debug=False,
        enable_asserts=False,
        target_bir_lowering=False,
        num_devices=NCORES,
        num_swdge_queues=4,
        dynamic_dma_scratch_size=K_SCR,
    )
    f32 = mybir.dt.float32
    f16 = mybir.dt.float16
    i16 = mybir.dt.int16
    u8 = mybir.dt.uint8
    tdt = f16 if TABLE_F16 else mybir.dt.int8
    add_op = mybir.AluOpType.add
    mult_op = mybir.AluOpType.mult

    table = nc.dram_tensor("table", [FEAT, E], tdt, kind="ExternalInput").ap()
    mega = nc.dram_tensor("mega", [P, MB], u8, kind="ExternalInput").ap()
    out = nc.dram_tensor("out", [P, NT], f32, kind="ExternalOutput").ap()

    mega_sb = nc.alloc_sbuf_tensor("mega_sb", [P, MB], u8).ap()
    idx_sb = mega_sb[:, MB_IDX:MB_IDX + 1664].bitcast(i16)        # [128, 832]
    dnt_sb = mega_sb[0:N_DENSE + 1, MB_DNT:MB_DNT + 2048].bitcast(f32)   # [14, 512]
    vdx_sb = mega_sb[0:N_DENSE + 1, MB_VDX:MB_VDX + 1256].bitcast(f32)   # [14, 314]

    g_sb = [
        [nc.alloc_sbuf_tensor(f"g{t}_{h}", [P, 13 * E], tdt).ap() for h in range(2)]
        for t in range(NT)
    ]
    # two alternating fp16 partial-sum sets (vector-engine-private; explicit
    # vv sems serialize same-engine RAW reuse); halves share one tensor so
    # the L2 level runs as a single 4-dim-AP op
    a_sb = [
        nc.alloc_sbuf_tensor(f"a{s}", [P, 12 * DP], f16).ap() for s in range(2)
    ]
    u_sb = [nc.alloc_sbuf_tensor(f"u{s}", [P, DP], f16).ap() for s in range(2)]
    # Act-converted fp16 copy of h0 rows 0..11 (two alternating buffers)
    gh_sb = [nc.alloc_sbuf_tensor(f"gh{s}", [P, 12 * DP], f16).ap() for s in range(2)]
    dnt16_sb = nc.alloc_sbuf_tensor("dnt16", [P, NT * DP], f16).ap()
    tot_sb = [nc.alloc_sbuf_tensor(f"tot{t}", [P, DP], f16).ap() for t in range(NT)]
    s8_sb = [nc.alloc_sbuf_tensor(f"s8_{t}", [P, K], f32).ap() for t in range(NT)]
    sq8_sb = [nc.alloc_sbuf_tensor(f"sq8_{s}", [P, K], f32).ap() for s in range(2)]
    sq_sb = nc.alloc_sbuf_tensor("sq", [P, D], f16).ap()
    # fp32 scratch for the DVE-side h1 squares (tot'^2 overflows fp16)
    sqd_sb = nc.alloc_sbuf_tensor("sqd", [P, D], f32).ap()
    h1_sb = [nc.alloc_sbuf_tensor(f"h1_{t}", [P, 1], f32).ap() for t in range(NT)]
    h2_sb = [nc.alloc_sbuf_tensor(f"h2_{t}", [P, 1], f32).ap() for t in range(NT)]
    rr_sb = [nc.alloc_sbuf_tensor(f"rr_{t}", [P, 1], f32).ap() for t in range(NT)]
    ot4_sb = nc.alloc_sbuf_tensor("ot4", [P, NT], f32).ap()
    ps_ps = [nc.alloc_psum_tensor(f"ps{t}", [P, DP], f32).ap() for t in range(NT)]

    io = nc.alloc_semaphore("io")      # mega load x 16
    st = nc.alloc_semaphore("st")      # output store x 16
    # one sem per sub-gather: a DMA sem may only be updated from one SWDGE
    # queue, so the sub-gathers of a tile-half can't share one
    gs = [
        [[nc.alloc_semaphore(f"gs{t}_{h}_{k}") for k in range(len(SPLITS_T[t]))]
         for h in range(2)]
        for t in range(NT)
    ]
    mm = nc.alloc_semaphore("mm")      # dense matmul done (per tile)
    ac = nc.alloc_semaphore("ac")      # Act PSUM->fp16 copy done (per tile)
    ah = nc.alloc_semaphore("ah")      # Act h1 accum done (per tile)
    av = nc.alloc_semaphore("av")      # Act h0 int8->fp16 convert done (per tile)
    dn = nc.alloc_semaphore("dn")      # ot column ready (per tile)
    vv = nc.alloc_semaphore("vv")      # vector-engine same-engine RAW ordering

    # vv increments per tile are TILE_VOPS (the final out op increments dn)

    with nc.Block() as block:

        @block.sync
        def _(sync):
            sync.dma_start(mega_sb[:], mega[:]).then_inc(io, 16)
            sync.wait_ge(dn, NT)
            sync.dma_start(out[:], ot4_sb[:]).then_inc(st, 16)
            sync.wait_ge(st, 16)

        @block.gpsimd
        def _(gpsimd):
            from concourse import library_config as lc

            gpsimd.load_library(lc.mlp)
            gpsimd.wait_ge(io, 16)
            for t, h, k, q in CALL_SCHED:
                c0, nf = SPLITS_T[t][k]
                g3 = g_sb[t][h].rearrange("p (c e) -> p c e", e=E)
                col = (2 * t + h) * HC
                gpsimd.dma_gather(
                    g3[:, c0:c0 + nf, :],
                    table[:],
                    idx_sb[:, col + c0 * 8:col + (c0 + nf) * 8],
                    nf * P,
                    nf * P,
                    E,
                    single_packet=False,
                    queue_num=q,
                ).then_inc(gs[t][h][k], 16)

        @block.tensor
        def _(tensor):
            tensor.wait_ge(io, 16)
            for t in range(NT):
                nc.tensor.matmul(
                    out=ps_ps[t][:],
                    lhsT=dnt_sb[:, t * P:(t + 1) * P],
                    rhs=vdx_sb[:],
                    start=True,
                    stop=True,
                ).then_inc(mm, 1)

        @block.scalar
        def _(scalar):
            copyf = mybir.ActivationFunctionType.Copy
            sqf = mybir.ActivationFunctionType.Square
            # downcast the dense-in-1/step-units PSUM to fp16 so the vector
            # add stays in the all-SBUF 2-byte fast path
            for t in range(NT):
                scalar.wait_ge(mm, t + 1)
                nc.scalar.activation(
                    out=dnt16_sb[:, t * DP:(t + 1) * DP], in_=ps_ps[t][:],
                    func=copyf,
                ).then_inc(ac, 1)
            # conv(t): convert h0 rows 0..11 to fp16 (two halves, so the
            # h1' accum of the previous tile can slot between them) so
            # DVE's L1h0 runs in the 2x fp16 path. Tile 0 skips conv (its
            # L1h0 runs int8 directly - the conv couldn't hide that early).
            def conva(t):
                g3 = g_sb[t][0].rearrange("p (c e) -> p c e", e=E)
                # rows 0..5 live in call 0 of either split scheme
                scalar.wait_ge(gs[t][0][0], 16)
                if t >= 3:
                    # buffer reuse: DVE must be done reading gh[t-2]
                    scalar.wait_ge(vv, TBASE[t - 2] + 2)
                nc.scalar.activation(
                    out=gh_sb[t % 2][:, 0:6 * DP], in_=g3[:, 0:6, 0:DP],
                    func=copyf,
                )

            def convb(t):
                g3 = g_sb[t][0].rearrange("p (c e) -> p c e", e=E)
                for k in L1SET_T[t]:
                    scalar.wait_ge(gs[t][0][k], 16)
                nc.scalar.activation(
                    out=gh_sb[t % 2][:, 6 * DP:12 * DP], in_=g3[:, 6:12, 0:DP],
                    func=copyf,
                ).then_inc(av, 1)

            def h1(t):
                scalar.wait_ge(vv, TBASE[t] + 8)
                nc.scalar.activation(
                    out=sq_sb[:], in_=tot_sb[t][:, :D],
                    func=sqf, accum_out=h1_sb[t][:],
                ).then_inc(ah, 1)

            if K_CONV:
                conva(1)
                convb(1)
                if DVE_H1 <= 0:
                    h1(0)
                conva(2)
                if DVE_H1 <= 1:
                    h1(1)
                convb(2)
                conva(3)
                if DVE_H1 <= 2:
                    h1(2)
                convb(3)
                if DVE_H1 <= 3:
                    h1(3)
            else:
                for t in range(DVE_H1, NT):
                    h1(t)
            scalar.wait_ge(st, 16)

        @block.vector
        def _(vector):
            def tadd(out, in0, in1):
                return nc.vector.tensor_tensor(out=out, in0=in0, in1=in1, op=add_op)

            for t in range(NT):
                base = TBASE[t]
                s = t % 2
                g03 = g_sb[t][0].rearrange("p (c e) -> p c e", e=E)
                g13 = g_sb[t][1].rearrange("p (c e) -> p c e", e=E)
                # a4[p, h, c, :]: h0 blocks at h=0, h1 blocks at h=1
                a4 = a_sb[s].rearrange("p (b c e) -> p b c e", b=2, e=DP)
                gh3 = gh_sb[s].rearrange("p (c e) -> p c e", e=DP)
                # ops 1-2: L1 pair adds. Tile 0 has no Act convert (the
                # conv can't finish before DVE needs h0 that early - it
                # measured as a net stall); later tiles do int8 h1 first so
                # the Act-converted fp16 h0 (2x path) hides behind it.
                if not K_CONV or t == 0:
                    for h in (0, 1):
                        for k in L1SET_T[t]:
                            vector.wait_ge(gs[t][h][k], 16)
                        gx = g03 if h == 0 else g13
                        tadd(a4[:, h, 0:6, :], gx[:, 0:6, 0:DP],
                             gx[:, 6:12, 0:DP]).then_inc(vv, 1)
                else:
                    for k in L1SET_T[t]:
                        vector.wait_ge(gs[t][1][k], 16)
                    tadd(a4[:, 1, 0:6, :], g13[:, 0:6, 0:DP],
                         g13[:, 6:12, 0:DP]).then_inc(vv, 1)
                    vector.wait_ge(av, t)
                    tadd(a4[:, 0, 0:6, :], gh3[:, 0:6, :],
                         gh3[:, 6:12, :]).then_inc(vv, 1)
                # op 3: L2 for both halves in one 4-dim-AP op
                vector.wait_ge(vv, base + 2)
                tadd(a4[:, :, 0:3, :], a4[:, :, 0:3, :],
                     a4[:, :, 3:6, :]).then_inc(vv, 1)
                # op 4: cross-half add into h0 blocks 0..2 (before the
                # 13th-row op so the tail only waits on the late last call)
                vector.wait_ge(vv, base + 3)
                tadd(a4[:, 0, 0:3, :], a4[:, 0, 0:3, :],
                     a4[:, 1, 0:3, :]).then_inc(vv, 1)
                # op 5: 13th rows of both halves folded into h0 block 3
                vector.wait_ge(vv, base + 3)
                vector.wait_ge(gs[t][0][ROW12CALL_T[t]], 16)
                vector.wait_ge(gs[t][1][ROW12CALL_T[t]], 16)
                tadd(a4[:, 0, 3, :], g03[:, 12, 0:DP],
                     g13[:, 12, 0:DP]).then_inc(vv, 1)
                # op 6: pairwise collapse of the 4 blocks, strided pairs in one op
                vector.wait_ge(vv, base + 5)
                tadd(a4[:, 0, 0:4:2, :], a4[:, 0, 0:4:2, :],
                     a4[:, 0, 1:4:2, :]).then_inc(vv, 1)
                # op 7: gathered total u = b0 + b2 (1/step units)
                vector.wait_ge(vv, base + 6)
                tadd(u_sb[s][:], a4[:, 0, 0, :], a4[:, 0, 2, :]).then_inc(vv, 1)
                # op 8: tot' = u + dnt'  (both already in 1/step units)
                vector.wait_ge(vv, base + 7)
                vector.wait_ge(ac, t + 1)
                tadd(tot_sb[t][:], u_sb[s][:],
                     dnt16_sb[:, t * DP:(t + 1) * DP]).then_inc(vv, 1)
                # op 9: s'_k = sum_i f'_ik   (Act computes h1' in parallel).
                # The table is laid out k-major so this reduce reads
                # contiguous runs of 39 (2x fp16 path).
                tv = tot_sb[t][:, :D].rearrange("p (k i) -> p k i", k=K)
                vector.wait_ge(vv, base + 8)
                nc.vector.reduce_sum(
                    out=s8_sb[t][:], in_=tv, axis=mybir.AxisListType.X
                ).then_inc(vv, 1)
                # ops 10-11: h2' = sum_k s'_k^2
                vector.wait_ge(vv, base + 9)
                nc.vector.tensor_tensor(
                    out=sq8_sb[s][:], in0=s8_sb[t][:], in1=s8_sb[t][:],
                    op=mult_op,
                ).then_inc(vv, 1)
                vector.wait_ge(vv, base + 10)
                nc.vector.reduce_sum(
                    out=h2_sb[t][:], in_=sq8_sb[s][:],
                    axis=mybir.AxisListType.X,
                ).then_inc(vv, 1)
                if t < DVE_H1:
                    # ops 12-13: h1' computed on DVE (early in the pipe the
                    # Act round-trip would stall the r op); fp32 out - the
                    # squared 1/step-unit values overflow fp16
                    vector.wait_ge(vv, base + 8)
                    nc.vector.tensor_tensor(
                        out=sqd_sb[:], in0=tot_sb[t][:, :D],
                        in1=tot_sb[t][:, :D], op=mult_op,
                    ).then_inc(vv, 1)
                    vector.wait_ge(vv, base + 12)
                    nc.vector.reduce_sum(
                        out=h1_sb[t][:], in_=sqd_sb[:],
                        axis=mybir.AxisListType.X,
                    ).then_inc(vv, 1)
                    nr = base + 13
                else:
                    vector.wait_ge(ah, t - DVE_H1 + 1)
                    nr = base + 11
                # r = (h2' - h1') * 0.5*step^2
                vector.wait_ge(vv, nr)
                nc.vector.tensor_scalar(
                    out=rr_sb[t][:], in0=h2_sb[t][:], scalar1=h1_sb[t][:],
                    scalar2=0.5 * step * step, op0=mybir.AluOpType.subtract,
                    op1=mult_op,
                ).then_inc(vv, 1)
                # final: out = tot'[312]*step + r
                vector.wait_ge(vv, nr + 1)
                nc.vector.tensor_scalar(
                    out=ot4_sb[:, t:t + 1], in0=tot_sb[t][:, D:DW],
                    scalar1=float(step), scalar2=rr_sb[t][:],
                    op0=mult_op, op1=add_op,
                ).then_inc(dn, 1)

    nc.compile()
    _cached[key] = nc
    return nc


def _quant_step(v, w):
    if TABLE_F16:
        return 1.0
    vflat = np.asarray(v, np.float32).reshape(FEAT, D)
    sigma = float(vflat[N_DENSE:].std())
    return sigma * CLIP_SIGMA / 127.0


def _prepare_inputs(inputs, w0, w, v, step):
    dense = np.ascontiguousarray(inputs[:, :N_DENSE].astype(np.float32))
    idx = inputs[:, N_DENSE:].astype(np.int32)
    flat_idx = (N_DENSE + np.arange(N_SPARSE, dtype=np.int32) * ONEHOT)[None, :] + idx

    # k-major layout (v[f, i, k] -> columns k*FIELD+i) so the kernel's
    # per-k reduce reads contiguous runs of FIELD
    vflat = np.ascontiguousarray(
        np.asarray(v, np.float32).transpose(0, 2, 1).reshape(FEAT, D)
    )
    wflat = np.asarray(w, np.float32).reshape(FEAT)
    if TABLE_F16:
        table = np.zeros((FEAT, E), np.float16)
        table[:, :D] = vflat.astype(np.float16)
        table[:, D] = wflat.astype(np.float16)
    else:
        table = np.zeros((FEAT, E), np.int8)
        table[:, :D] = np.clip(np.rint(vflat / step), -127, 127).astype(np.int8)
        table[:, D] = np.clip(np.rint(wflat / step), -127, 127).astype(np.int8)

    # dense-side v/w/w0 in 1/step units so the gathered int sums add directly
    w0_row = np.zeros((1, DP), np.float32)
    w0_row[0, D] = np.asarray(w0, np.float32).reshape(-1)[0] / step
    vdx_top = np.concatenate(
        [vflat[:N_DENSE] / step, wflat[:N_DENSE, None] / step,
         np.zeros((N_DENSE, 1), np.float32)],
        axis=1,
    ).astype(np.float32)
    vdx = np.ascontiguousarray(np.concatenate([vdx_top, w0_row], axis=0))

    in_maps = []
    for c in range(NCORES):
        sl = slice(c * BC, (c + 1) * BC)
        dnt = np.concatenate(
            [dense[sl].T, np.ones((1, BC), np.float32)], axis=0
        )  # [14, 512]
        # per tile t and half h the gather consumes indices i = f_local*128+p,
        # laid out int16 at [i % 16, i // 16] in the first 16 partitions,
        # replicated 8x down the partitions (one copy per Q7 core)
        fi = flat_idx[sl].astype(np.int16)  # [512, 26]
        blocks = []
        for t in range(NT):
            for h in range(2):
                lin = fi[t * P:(t + 1) * P, 13 * h:13 * (h + 1)].T.reshape(NI // 2)
                blk = lin.reshape(NI // 32, 16).T  # [16, HC]
                blocks.append(np.tile(blk, (8, 1)))  # [128, HC]
        idx_buf = np.ascontiguousarray(np.concatenate(blocks, axis=1))

        mega = np.zeros((P, MB), np.uint8)
        mega[:, MB_IDX:MB_IDX + 1664] = idx_buf.view(np.uint8)
        mega[0:N_DENSE + 1, MB_DNT:MB_DNT + 2048] = (
            np.ascontiguousarray(dnt).view(np.uint8)
        )
        mega[0:N_DENSE + 1, MB_VDX:MB_VDX + 1256] = vdx.view(np.uint8)
        in_maps.append({"table": table, "mega": mega})
    return in_maps


def kernel(**inputs):
    from concourse import bass_utils

    v = np.asarray(inputs["v"])
    w = np.asarray(inputs["w"])
    step = _quant_step(v, w)
    nc = _build_program(step)
    in_maps = _prepare_inputs(
        np.asarray(inputs["inputs"]),
        np.asarray(inputs["w0"]),
        w,
        v,
        step,
    )
    res = bass_utils.run_bass_kernel_spmd(nc, in_maps, core_ids=list(range(NCORES)))
    outs = [
        np.asarray(res.results[c]["out"]).T.reshape(BC, 1) for c in range(NCORES)
    ]
    return np.concatenate(outs, axis=0).astype(np.float32)


# revision 59
# speedup vs baseline: 1.0388x; 1.0388x over previous
"""FFM layer kernel for Trainium2, data-parallel over batch on 8 NeuronCores.

The reference computes, for each sample b:
    x = [dense(13) | onehot(26 fields x 1000)]            # [B, 26013]
    linear = w0 + x @ w                                   # [B, 1]
    field_f = einsum('bf,fik->bik', x, v)                 # [B, 39, 8]
    inter = 0.5*((sum_i field_f)^2.sum(k) - (field_f^2).sum(i,k))
    out = linear + inter

Because x is one-hot in the sparse block, x @ [v|w] is a 26-row gather from
an int8 [26013, 512] table (cols 0..311 = k-major flattened v row / step,
col 312 = w / step, rest pad to a 256 B multiple) plus a tiny fp32 dense
[14]x[14,314] matmul in 1/step units (vdx pre-divided by step on the host;
row 13 = ones row carrying w0/step into col 312).  Each core handles 512
samples as 4 tiles of 128.

Measured HW behavior that shaped the design: SWDGE desc-gen costs ~8.5 ns
per descriptor (+~1 us fixed per call) with ~4 concurrent per-queue lanes,
and a call's packets only flow once ITS generation completes - so the
gather is desc-GEN limited, not drain limited, and the call schedule
(CALL_SCHED) hand-balances fields per queue with tile 0 on all 4 lanes in
round 1.  Startup is dominated by the gpsimd mlp-library reload (~10.6 us,
fixed) which gates the first gather; the mega input DMA and dense matmuls
hide under it.  The int8 table (4-sigma clipped step; rel_fro ~1.1e-2 vs
the 2e-2 gate) keeps wire bytes at 512 B/row vs fp16's 768 B.

The 26 gathered rows are summed with a pairwise tree on DVE.  Level-1 of
half 0 is pre-converted int8->fp16 by the otherwise-idle Act engine (TRN2
DVE reads int8 at ~0.9 elem/ns vs fp16 2x ~1.65), half 1 runs int8 directly
(first, so the convert hides behind it); upper levels run fp16 2x.
Everything stays in integer "1/step" units until the very end: tot' = u +
dnt', h1' = act-square-accum(tot'), r = (h2'-h1') * 0.5*step^2, out =
tot'[312]*step + r.  The [128, 4] per-tile outputs are stored directly and
un-permuted on the host (no PE transpose pass).

Measured: 67.7-68.5 us warm (from an 86.1 us fp16 baseline; cold first
exec after a fresh compile reads several us higher). Structure: reload
~17.6 + gather-gen-limited window + gapless DVE ~32.5 us from ~29.5 +
store ~3.7.
"""

import os

import numpy as np

N_DENSE = 13
N_SPARSE = 26
ONEHOT = 1000
FIELD = 39
K = 8
FEAT = N_DENSE + N_SPARSE * ONEHOT  # 26013
B = 4096
NCORES = 8
BC = B // NCORES  # 512 samples per core
P = 128
NT = BC // P  # 4 tiles per core
D = FIELD * K  # 312
DW = D + 1  # 313 (col 312 carries the linear weight)
DP = D + 2  # 314 (even payload width; col 313 zero pad)
NI = N_SPARSE * P  # 3328 gathered rows per tile
HC = (NI // 2) // 16  # idx columns per 13-field half (104)

TABLE_F16 = os.environ.get("K_F16", "0") == "1"
CLIP_SIGMA = float(os.environ.get("K_CLIP", "4.0"))
K_SCR = int(os.environ.get("K_SCR", "65536"))  # SWDGE desc ring carveout
E = 384 if TABLE_F16 else 512  # gathered row width in elements (=bytes for i8)
# SWDGE desc-gen runs at ~8.5 ns/descriptor + ~1 us fixed per call, with 4
# concurrent per-queue lanes; a call's packets only flow once ITS generation
# finishes. (7,6) per half for tiles 0-2 puts tile 0 on all 4 lanes at once
# (early DVE start) and minimizes per-call fixed cost; the LAST tile uses
# (6,6,1) so its L1 pair-adds only need the two 6-field calls and the lone
# 13th field feeds a late op in the tree - the post-gather tail chain
# shrinks from ~7 us to ~3.5 us.
SPLITS_T = (
    [((0, 5), (5, 4), (9, 4))]
    + [((0, 7), (7, 6))] * 2
    + [((0, 6), (6, 6), (12, 1))]
)
# calls whose rows feed the L1 pair adds (rows 0..11)
L1SET_T = [(0, 1, 2)] + [(0, 1)] * 3
ROW12CALL_T = [2, 1, 1, 2]  # call index holding field 12, per tile
# explicit (tile, half, call) issue order with hand-balanced queues
# (26 fields per queue, dispatch order matches lane-free order). Tile 0 is
# fine-split (5,4,4) so its six calls pipeline through the lanes and the
# DVE can start ~1.4 us earlier; the schedule tail is fine-grained -
# ending with a full coarse round measured +3.8 us of window.
CALL_SCHED = [
    (0, 0, 0, 0), (0, 1, 0, 1), (0, 0, 1, 2), (0, 1, 1, 3),
    (0, 0, 2, 2), (0, 1, 2, 3),
    (1, 0, 0, 0), (1, 1, 0, 1), (1, 0, 1, 2), (1, 1, 1, 3),
    (2, 0, 0, 0), (2, 1, 0, 1), (2, 0, 1, 2), (2, 1, 1, 3),
    (3, 0, 0, 0), (3, 1, 0, 1), (3, 0, 1, 2), (3, 1, 1, 3),
    (3, 0, 2, 0), (3, 1, 2, 1),
]
K_CONV = os.environ.get("K_CONV", "1") == "1"  # Act converts h0 rows to fp16
# h1 runs on DVE for the first DVE_H1 tiles, on Act for the rest.
# DVE_H1=2 should kill the early-tile r-op stalls but measured net-slower
# (71.7us vs 69.3us); 0 is the measured best.
DVE_H1 = int(os.environ.get("K_DVEH1", "0"))
TILE_VOPS = [14 if t < DVE_H1 else 12 for t in range(NT)]
TBASE = [sum(TILE_VOPS[:t]) for t in range(NT)]

# mega input layout (bytes per partition)
MB_IDX = 0                    # [128, 832] int16 = 1664 B
MB_DNT = 1664                 # [14, 512] fp32 = 2048 B
MB_VDX = MB_DNT + 2048        # [14, 314] fp32 = 1256 B
MB = MB_VDX + 1256            # 4968

_cached = {}


def _build_program(step: float):
    key = step
    if key in _cached:
        return _cached[key]

    import concourse.bacc as bacc
    import concourse.mybir as mybir

    nc = bacc.Bacc(
        "TRN2",
        debug=False,
        enable_asserts=False,
        target_bir_lowering=False,
        num_devices=NCORES,
        num_swdge_queues=4,
        dynamic_dma_scratch_size=K_SCR,
    )
    f32 = mybir.dt.float32
    f16 = mybir.dt.float16
    i16 = mybir.dt.int16
    u8 = mybir.dt.uint8
    tdt = f16 if TABLE_F16 else mybir.dt.int8
    add_op = mybir.AluOpType.add
    mult_op = mybir.AluOpType.mult

    table = nc.dram_tensor("table", [FEAT, E], tdt, kind="ExternalInput").ap()
    mega = nc.dram_tensor("mega", [P, MB], u8, kind="ExternalInput").ap()
    out = nc.dram_tensor("out", [P, NT], f32, kind="ExternalOutput").ap()

    mega_sb = nc.alloc_sbuf_tensor("mega_sb", [P, MB], u8).ap()
    idx_sb = mega_sb[:, MB_IDX:MB_IDX + 1664].bitcast(i16)        # [128, 832]
    dnt_sb = mega_sb[0:N_DENSE + 1, MB_DNT:MB_DNT + 2048].bitcast(f32)   # [14, 512]
    vdx_sb = mega_sb[0:N_DENSE + 1, MB_VDX:MB_VDX + 1256].bitcast(f32)   # [14, 314]

    g_sb = [
        [nc.alloc_sbuf_tensor(f"g{t}_{h}", [P, 13 * E], tdt).ap() for h in range(2)]
        for t in range(NT)
    ]
    # two alternating fp16 partial-sum sets (vector-engine-private; explicit
    # vv sems serialize same-engine RAW reuse); halves share one tensor so
    # the L2 level runs as a single 4-dim-AP op
    a_sb = [
        nc.alloc_sbuf_tensor(f"a{s}", [P, 12 * DP], f16).ap() for s in range(2)
    ]
    u_sb = [nc.alloc_sbuf_tensor(f"u{s}", [P, DP], f16).ap() for s in range(2)]
    # Act-converted fp16 copy of h0 rows 0..11 (two alternating buffers)
    gh_sb = [nc.alloc_sbuf_tensor(f"gh{s}", [P, 12 * DP], f16).ap() for s in range(2)]
    dnt16_sb = nc.alloc_sbuf_tensor("dnt16", [P, NT * DP], f16).ap()
    tot_sb = [nc.alloc_sbuf_tensor(f"tot{t}", [P, DP], f16).ap() for t in range(NT)]
    s8_sb = [nc.alloc_sbuf_tensor(f"s8_{t}", [P, K], f32).ap() for t in range(NT)]
    sq8_sb = [nc.alloc_sbuf_tensor(f"sq8_{s}", [P, K], f32).ap() for s in range(2)]
    sq_sb = nc.alloc_sbuf_tensor("sq", [P, D], f16).ap()
    # fp32 scratch for the DVE-side h1 squares (tot'^2 overflows fp16)
    sqd_sb = nc.alloc_sbuf_tensor("sqd", [P, D], f32).ap()
    h1_sb = [nc.alloc_sbuf_tensor(f"h1_{t}", [P, 1], f32).ap() for t in range(NT)]
    h2_sb = [nc.alloc_sbuf_tensor(f"h2_{t}", [P, 1], f32).ap() for t in range(NT)]
    rr_sb = [nc.alloc_sbuf_tensor(f"rr_{t}", [P, 1], f32).ap() for t in range(NT)]
    ot4_sb = nc.alloc_sbuf_tensor("ot4", [P, NT], f32).ap()
    ps_ps = [nc.alloc_psum_tensor(f"ps{t}", [P, DP], f32).ap() for t in range(NT)]

    io = nc.alloc_semaphore("io")      # mega load x 16
    st = nc.alloc_semaphore("st")      # output store x 16
    # one sem per sub-gather: a DMA sem may only be updated from one SWDGE
    # queue, so the sub-gathers of a tile-half can't share one
    gs = [
        [[nc.alloc_semaphore(f"gs{t}_{h}_{k}") for k in range(len(SPLITS_T[t]))]
         for h in range(2)]
        for t in range(NT)
    ]
    mm = nc.alloc_semaphore("mm")      # dense matmul done (per tile)
    ac = nc.alloc_semaphore("ac")      # Act PSUM->fp16 copy done (per tile)
    ah = nc.alloc_semaphore("ah")      # Act h1 accum done (per tile)
    av = nc.alloc_semaphore("av")      # Act h0 int8->fp16 convert done (per tile)
    dn = nc.alloc_semaphore("dn")      # ot column ready (per tile)
    vv = nc.alloc_semaphore("vv")      # vector-engine same-engine RAW ordering

    # vv increments per tile are TILE_VOPS (the final out op increments dn)

    with nc.Block() as block:

        @block.sync
        def _(sync):
            sync.dma_start(mega_sb[:], mega[:]).then_inc(io, 16)
            sync.wait_ge(dn, NT)
            sync.dma_start(out[:], ot4_sb[:]).then_inc(st, 16)
            sync.wait_ge(st, 16)

        @block.gpsimd
        def _(gpsimd):
            from concourse import library_config as lc

            gpsimd.load_library(lc.mlp)
            gpsimd.wait_ge(io, 16)
            for t, h, k, q in CALL_SCHED:
                c0, nf = SPLITS_T[t][k]
                g3 = g_sb[t][h].rearrange("p (c e) -> p c e", e=E)
                col = (2 * t + h) * HC
                gpsimd.dma_gather(
                    g3[:, c0:c0 + nf, :],
                    table[:],
                    idx_sb[:, col + c0 * 8:col + (c0 + nf) * 8],
                    nf * P,
                    nf * P,
                    E,
                    single_packet=False,
                    queue_num=q,
                ).then_inc(gs[t][h][k], 16)

        @block.tensor
        def _(tensor):
            tensor.wait_ge(io, 16)
            for t in range(NT):
                nc.tensor.matmul(
                    out=ps_ps[t][:],
                    lhsT=dnt_sb[:, t * P:(t + 1) * P],
                    rhs=vdx_sb[:],
                    start=True,
                    stop=True,
                ).then_inc(mm, 1)

        @block.scalar
        def _(scalar):
            copyf = mybir.ActivationFunctionType.Copy
            sqf = mybir.ActivationFunctionType.Square
            # downcast the dense-in-1/step-units PSUM to fp16 so the vector
            # add stays in the all-SBUF 2-byte fast path
            for t in range(NT):
                scalar.wait_ge(mm, t + 1)
                nc.scalar.activation(
                    out=dnt16_sb[:, t * DP:(t + 1) * DP], in_=ps_ps[t][:],
                    func=copyf,
                ).then_inc(ac, 1)
            # conv(t): convert h0 rows 0..11 to fp16 (two halves, so the
            # h1' accum of the previous tile can slot between them) so
            # DVE's L1h0 runs in the 2x fp16 path. Tile 0 skips conv (its
            # L1h0 runs int8 directly - the conv couldn't hide that early).
            def conva(t):
                g3 = g_sb[t][0].rearrange("p (c e) -> p c e", e=E)
                # rows 0..5 live in call 0 of either split scheme
                scalar.wait_ge(gs[t][0][0], 16)
                if t >= 3:
                    # buffer reuse: DVE must be done reading gh[t-2]
                    scalar.wait_ge(vv, TBASE[t - 2] + 2)
                nc.scalar.activation(
                    out=gh_sb[t % 2][:, 0:6 * DP], in_=g3[:, 0:6, 0:DP],
                    func=copyf,
                )

            def convb(t):
                g3 = g_sb[t][0].rearrange("p (c e) -> p c e", e=E)
                for k in L1SET_T[t]:
                    scalar.wait_ge(gs[t][0][k], 16)
                nc.scalar.activation(
                    out=gh_sb[t % 2][:, 6 * DP:12 * DP], in_=g3[:, 6:12, 0:DP],
                    func=copyf,
                ).then_inc(av, 1)

            def h1(t):
                scalar.wait_ge(vv, TBASE[t] + 8)
                nc.scalar.activation(
                    out=sq_sb[:], in_=tot_sb[t][:, :D],
                    func=sqf, accum_out=h1_sb[t][:],
                ).then_inc(ah, 1)

            if K_CONV:
                conva(1)
                # h1(0) BEFORE convb1: its deq(t0) dependency lands mid-conv1
                # and the ~1.4us r(t0) stall beats convb1's ~1us slack to
                # DVE's op2(t1)
                if DVE_H1 <= 0:
                    h1(0)
                convb(1)
                conva(2)
                if DVE_H1 <= 1:
                    h1(1)
                convb(2)
                conva(3)
                if DVE_H1 <= 2:
                    h1(2)
                convb(3)
                if DVE_H1 <= 3:
                    h1(3)
            else:
                for t in range(DVE_H1, NT):
                    h1(t)
            scalar.wait_ge(st, 16)

        @block.vector
        def _(vector):
            def tadd(out, in0, in1):
                return nc.vector.tensor_tensor(out=out, in0=in0, in1=in1, op=add_op)

            for t in range(NT):
                base = TBASE[t]
                s = t % 2
                g03 = g_sb[t][0].rearrange("p (c e) -> p c e", e=E)
                g13 = g_sb[t][1].rearrange("p (c e) -> p c e", e=E)
                # a4[p, h, c, :]: h0 blocks at h=0, h1 blocks at h=1
                a4 = a_sb[s].rearrange("p (b c e) -> p b c e", b=2, e=DP)
                gh3 = gh_sb[s].rearrange("p (c e) -> p c e", e=DP)
                # ops 1-2: L1 pair adds. Tile 0 has no Act convert (the
                # conv can't finish before DVE needs h0 that early - it
                # measured as a net stall); later tiles do int8 h1 first so
                # the Act-converted fp16 h0 (2x path) hides behind it.
                if not K_CONV or t == 0:
                    for h in (0, 1):
                        for k in L1SET_T[t]:
                            vector.wait_ge(gs[t][h][k], 16)
                        gx = g03 if h == 0 else g13
                        tadd(a4[:, h, 0:6, :], gx[:, 0:6, 0:DP],
                             gx[:, 6:12, 0:DP]).then_inc(vv, 1)
                else:
                    for k in L1SET_T[t]:
                        vector.wait_ge(gs[t][1][k], 16)
                    tadd(a4[:, 1, 0:6, :], g13[:, 0:6, 0:DP],
                         g13[:, 6:12, 0:DP]).then_inc(vv, 1)
                    vector.wait_ge(av, t)
                    tadd(a4[:, 0, 0:6, :], gh3[:, 0:6, :],
                         gh3[:, 6:12, :]).then_inc(vv, 1)
                # op 3: L2 for both halves in one 4-dim-AP op
                vector.wait_ge(vv, base + 2)
                tadd(a4[:, :, 0:3, :], a4[:, :, 0:3, :],
                     a4[:, :, 3:6, :]).then_inc(vv, 1)
                # op 4: cross-half add into h0 blocks 0..2 (before the
                # 13th-row op so the tail only waits on the late last call)
                vector.wait_ge(vv, base + 3)
                tadd(a4[:, 0, 0:3, :], a4[:, 0, 0:3, :],
                     a4[:, 1, 0:3, :]).then_inc(vv, 1)
                # op 5: 13th rows of both halves folded into h0 block 3
                vector.wait_ge(vv, base + 3)
                vector.wait_ge(gs[t][0][ROW12CALL_T[t]], 16)
                vector.wait_ge(gs[t][1][ROW12CALL_T[t]], 16)
                tadd(a4[:, 0, 3, :], g03[:, 12, 0:DP],
                     g13[:, 12, 0:DP]).then_inc(vv, 1)
                # op 6: pairwise collapse of the 4 blocks, strided pairs in one op
                vector.wait_ge(vv, base + 5)
                tadd(a4[:, 0, 0:4:2, :], a4[:, 0, 0:4:2, :],
                     a4[:, 0, 1:4:2, :]).then_inc(vv, 1)
                # op 7: gathered total u = b0 + b2 (1/step units)
                vector.wait_ge(vv, base + 6)
                tadd(u_sb[s][:], a4[:, 0, 0, :], a4[:, 0, 2, :]).then_inc(vv, 1)
                # op 8: tot' = u + dnt'  (both already in 1/step units)
                vector.wait_ge(vv, base + 7)
                vector.wait_ge(ac, t + 1)
                tadd(tot_sb[t][:], u_sb[s][:],
                     dnt16_sb[:, t * DP:(t + 1) * DP]).then_inc(vv, 1)
                # op 9: s'_k = sum_i f'_ik   (Act computes h1' in parallel).
                # The table is laid out k-major so this reduce reads
                # contiguous runs of 39 (2x fp16 path).
                tv = tot_sb[t][:, :D].rearrange("p (k i) -> p k i", k=K)
                vector.wait_ge(vv, base + 8)
                nc.vector.reduce_sum(
                    out=s8_sb[t][:], in_=tv, axis=mybir.AxisListType.X
                ).then_inc(vv, 1)
                # ops 10-11: h2' = sum_k s'_k^2
                vector.wait_ge(vv, base + 9)
                nc.vector.tensor_tensor(
                    out=sq8_sb[s][:], in0=s8_sb[t][:], in1=s8_sb[t][:],
                    op=mult_op,
                ).then_inc(vv, 1)
                vector.wait_ge(vv, base + 10)
                nc.vector.reduce_sum(
                    out=h2_sb[t][:], in_=sq8_sb[s][:],
                    axis=mybir.AxisListType.X,
                ).then_inc(vv, 1)
                if t < DVE_H1:
                    # ops 12-13: h1' computed on DVE (early in the pipe the
                    # Act round-trip would stall the r op); fp32 out - the
                    # squared 1/step-unit values overflow fp16
                    vector.wait_ge(vv, base + 8)
                    nc.vector.tensor_tensor(
                        out=sqd_sb[:], in0=tot_sb[t][:, :D],
                        in1=tot_sb[t][:, :D], op=mult_op,
                    ).then_inc(vv, 1)
                    vector.wait_ge(vv, base + 12)
                    nc.vector.reduce_sum(
                        out=h1_sb[t][:], in_=sqd_sb[:],
                        axis=mybir.AxisListType.X,
                    ).then_inc(vv, 1)
                    nr = base + 13
                else:
                    vector.wait_ge(ah, t - DVE_H1 + 1)
                    nr = base + 11
                # r = (h2' - h1') * 0.5*step^2
                vector.wait_ge(vv, nr)
                nc.vector.tensor_scalar(
                    out=rr_sb[t][:], in0=h2_sb[t][:], scalar1=h1_sb[t][:],
                    scalar2=0.5 * step * step, op0=mybir.AluOpType.subtract,
                    op1=mult_op,
                ).then_inc(vv, 1)
                # final: out = tot'[312]*step + r
                vector.wait_ge(vv, nr + 1)
                nc.vector.tensor_scalar(
                    out=ot4_sb[:, t:t + 1], in0=tot_sb[t][:, D:DW],
                    scalar1=float(step), scalar2=rr_sb[t][:],
                    op0=mult_op, op1=add_op,
                ).then_inc(dn, 1)

    nc.compile()
    _cached[key] = nc
    return nc


def _quant_step(v, w):
    if TABLE_F16:
        return 1.0
    vflat = np.asarray(v, np.float32).reshape(FEAT, D)
    sigma = float(vflat[N_DENSE:].std())
    return sigma * CLIP_SIGMA / 127.0


def _prepare_inputs(inputs, w0, w, v, step):
    dense = np.ascontiguousarray(inputs[:, :N_DENSE].astype(np.float32))
    idx = inputs[:, N_DENSE:].astype(np.int32)
    flat_idx = (N_DENSE + np.arange(N_SPARSE, dtype=np.int32) * ONEHOT)[None, :] + idx

    # k-major layout (v[f, i, k] -> columns k*FIELD+i) so the kernel's
    # per-k reduce reads contiguous runs of FIELD
    vflat = np.ascontiguousarray(
        np.asarray(v, np.float32).transpose(0, 2, 1).reshape(FEAT, D)
    )
    wflat = np.asarray(w, np.float32).reshape(FEAT)
    if TABLE_F16:
        table = np.zeros((FEAT, E), np.float16)
        table[:, :D] = vflat.astype(np.float16)
        table[:, D] = wflat.astype(np.float16)
    else:
        table = np.zeros((FEAT, E), np.int8)
        table[:, :D] = np.clip(np.rint(vflat / step), -127, 127).astype(np.int8)
        table[:, D] = np.clip(np.rint(wflat / step), -127, 127).astype(np.int8)

    # dense-side v/w/w0 in 1/step units so the gathered int sums add directly
    w0_row = np.zeros((1, DP), np.float32)
    w0_row[0, D] = np.asarray(w0, np.float32).reshape(-1)[0] / step
    vdx_top = np.concatenate(
        [vflat[:N_DENSE] / step, wflat[:N_DENSE, None] / step,
         np.zeros((N_DENSE, 1), np.float32)],
        axis=1,
    ).astype(np.float32)
    vdx = np.ascontiguousarray(np.concatenate([vdx_top, w0_row], axis=0))

    in_maps = []
    for c in range(NCORES):
        sl = slice(c * BC, (c + 1) * BC)
        dnt = np.concatenate(
            [dense[sl].T, np.ones((1, BC), np.float32)], axis=0
        )  # [14, 512]
        # per tile t and half h the gather consumes indices i = f_local*128+p,
        # laid out int16 at [i % 16, i // 16] in the first 16 partitions,
        # replicated 8x down the partitions (one copy per Q7 core)
        fi = flat_idx[sl].astype(np.int16)  # [512, 26]
        blocks = []
        for t in range(NT):
            for h in range(2):
                lin = fi[t * P:(t + 1) * P, 13 * h:13 * (h + 1)].T.reshape(NI // 2)
                blk = lin.reshape(NI // 32, 16).T  # [16, HC]
                blocks.append(np.tile(blk, (8, 1)))  # [128, HC]
        idx_buf = np.ascontiguousarray(np.concatenate(blocks, axis=1))

        mega = np.zeros((P, MB), np.uint8)
        mega[:, MB_IDX:MB_IDX + 1664] = idx_buf.view(np.uint8)
        mega[0:N_DENSE + 1, MB_DNT:MB_DNT + 2048] = (
            np.ascontiguousarray(dnt).view(np.uint8)
        )
        mega[0:N_DENSE + 1, MB_VDX:MB_VDX + 1256] = vdx.view(np.uint8)
        in_maps.append({"table": table, "mega": mega})
    return in_maps


def kernel(**inputs):
    from concourse import bass_utils

    v = np.asarray(inputs["v"])
    w = np.asarray(inputs["w"])
    step = _quant_step(v, w)
    nc = _build_program(step)
    in_maps = _prepare_inputs(
        np.asarray(inputs["inputs"]),
        np.asarray(inputs["w0"]),
        w,
        v,
        step,
    )
    res = bass_utils.run_bass_kernel_spmd(nc, in_maps, core_ids=list(range(NCORES)))
    outs = [
        np.asarray(res.results[c]["out"]).T.reshape(BC, 1) for c in range(NCORES)
    ]
    return np.concatenate(outs, axis=0).astype(np.float32)
